# revision 3
# baseline (speedup 1.0000x reference)
"""Burgers PINN residual kernel for Trainium2 (8 NeuronCores, data parallel).

Computes, for an MLP u(t,x) with layers [2,128,128,128,128,1] (tanh hidden):
  - burgers_eq = u_t + u*u_x - NU*u_xx  on tx_equation  (131072 points)
  - u on tx_init / tx_bound                              (4096 + 4096 points)

Strategy: forward Taylor-mode propagation of 4 streams per point
(h, dh/dt, dh/dx, and hxx~ := -0.5 * d2h/dx2), feature-major layout
[128 features x 512 points] per chunk. The scaled-xx stream makes the
per-layer update  hxx~' = d*zxx~ + s*(d*zx^2)  (d = 1 - tanh^2(z)),
with no extra scale/subtract ops. The output layer uses masked-W4
stationary matrices so every chunk's scalar results land on a distinct
PSUM partition of a single accumulator bank:
  rows 0..31  : u            (chunk c -> row c)
  rows 32..63 : u_t - NU*u_xx  (two accumulating matmuls; the xx matmul
                               uses a 2*NU-scaled W4 column, since
                               W4^T hxx~ = -0.5 * u_xx)
  rows 64..95 : u_x
  rows 96..97 : u on tx_init / tx_bound
"""

import numpy as np

import concourse.bass as bass
import concourse.tile as tile
from concourse import bacc, mybir
from concourse.bass_utils import run_bass_kernel_spmd

# ---- problem constants (hardcoded per contract) ----
N_CORES = 8
N_EQ, N_INIT, N_BND = 131072, 4096, 4096
NU = float(0.01 / np.pi)

P = 128          # partitions / hidden width
CH = 512         # points per chunk
NEQC = N_EQ // N_CORES          # 16384 eq points per core
NCH_EQ = NEQC // CH             # 32 chunks
NIBC = N_INIT // N_CORES        # 512 init (and bound) points per core

F32 = mybir.dt.float32
F16 = mybir.dt.float16
AF = mybir.ActivationFunctionType
ALU = mybir.AluOpType


def _build_program():
    nc = bacc.Bacc("TRN2", target_bir_lowering=False, debug=False)

    tx_eq = nc.dram_tensor("tx_eq", [NEQC, 2], F32, kind="ExternalInput")
    tx_init = nc.dram_tensor("tx_init", [NIBC, 2], F32, kind="ExternalInput")
    tx_bound = nc.dram_tensor("tx_bound", [NIBC, 2], F32, kind="ExternalInput")
    W_dram = [
        nc.dram_tensor("W0", [2, P], F32, kind="ExternalInput"),
        nc.dram_tensor("W1", [P, P], F32, kind="ExternalInput"),
        nc.dram_tensor("W2", [P, P], F32, kind="ExternalInput"),
        nc.dram_tensor("W3", [P, P], F32, kind="ExternalInput"),
        nc.dram_tensor("W4", [P, 1], F32, kind="ExternalInput"),
    ]
    b_dram = [
        nc.dram_tensor("b0", [P], F32, kind="ExternalInput"),
        nc.dram_tensor("b1", [P], F32, kind="ExternalInput"),
        nc.dram_tensor("b2", [P], F32, kind="ExternalInput"),
        nc.dram_tensor("b3", [P], F32, kind="ExternalInput"),
        nc.dram_tensor("b4", [1], F32, kind="ExternalInput"),
    ]
    out_eq = nc.dram_tensor("out_eq", [NEQC, 1], F32, kind="ExternalOutput")
    out_init = nc.dram_tensor("out_init", [NIBC, 1], F32, kind="ExternalOutput")
    out_bound = nc.dram_tensor("out_bound", [NIBC, 1], F32, kind="ExternalOutput")

    with tile.TileContext(nc) as tc:
        with (
            tc.tile_pool(name="const", bufs=1) as const,
            tc.tile_pool(name="io", bufs=4) as io_pool,
            tc.tile_pool(name="acts", bufs=3) as acts,
            tc.tile_pool(name="tmp", bufs=3) as tmp,
            tc.tile_pool(name="psz", bufs=2, space="PSUM") as psum_z,
            tc.tile_pool(name="psd", bufs=1, space="PSUM") as psum_d,
            tc.tile_pool(name="psf", bufs=1, space="PSUM") as psum_f,
        ):
            # ---------- constants ----------
            w0_f32 = const.tile([2, P], F32, tag="w0")
            nc.sync.dma_start(out=w0_f32[:], in_=W_dram[0][:, :])

            wf16 = {}
            for layer in (1, 2, 3):
                w_f32 = const.tile([P, P], F32, tag=f"w{layer}_f32")
                nc.sync.dma_start(out=w_f32[:], in_=W_dram[layer][:, :])
                wf16[layer] = const.tile(
                    [P, P], F16, tag=f"w{layer}_f16", name=f"w{layer}_f16"
                )
                nc.vector.tensor_copy(out=wf16[layer][:], in_=w_f32[:])

            w4sb = const.tile([P, 1], F32, tag="w4")
            nc.sync.dma_start(out=w4sb[:], in_=W_dram[4][:, :])

            bv = {}
            for layer in range(4):
                bv[layer] = const.tile([P, 1], F32, tag=f"b{layer}", name=f"bv{layer}")
                nc.sync.dma_start(
                    out=bv[layer][:],
                    in_=b_dram[layer].rearrange("(p o) -> p o", o=1),
                )
            b4bc = const.tile([P, 1], F32, tag="b4bc")
            b4_ap = b_dram[4].rearrange("(p o) -> p o", o=1)  # [1,1]
            nc.sync.dma_start(out=b4bc[:], in_=b4_ap.to_broadcast([P, 1]))

            # W0 rows as per-partition vectors (tangent seeds)
            w0t = const.tile([P, 1], F32, tag="w0t")
            nc.sync.dma_start(out=w0t[:], in_=W_dram[0][0:1, :].rearrange("o p -> p o"))
            w0x = const.tile([P, 1], F32, tag="w0x")
            nc.sync.dma_start(out=w0x[:], in_=W_dram[0][1:2, :].rearrange("o p -> p o"))
            w0xsq = const.tile([P, 1], F32, tag="w0xsq")
            nc.vector.tensor_tensor(out=w0xsq[:], in0=w0x[:], in1=w0x[:], op=ALU.mult)

            # Masked-W4 pad buffer: zeros except col 128 = W4, col 256 = 2*NU*W4.
            # lhsT slice [128-m : 256-m] has W4 in column m (zeros elsewhere);
            # slice [256-m : 384-m] has 2*NU*W4 in column m.
            b4pad = const.tile([P, 3 * P], F16, tag="b4pad")
            nc.vector.memset(b4pad[:], 0.0)
            nc.vector.tensor_copy(out=b4pad[:, P : P + 1], in_=w4sb[:])
            nc.vector.tensor_scalar(
                b4pad[:, 2 * P : 2 * P + 1], w4sb[:], 2.0 * NU, None, ALU.mult
            )

            # persistent output accumulator (one PSUM bank)
            fin = psum_f.tile([P, CH], F32, tag="fin")

            def mlp_layer_forward(h, layer):
                """z = W h; return tanh(z + b) as fp16 tile."""
                zh = psum_z.tile([P, CH], F32, tag="zh")
                lhsT = w0_f32[:] if layer == 0 else wf16[layer][:]
                nc.tensor.matmul(zh[:], lhsT, h, start=True, stop=True)
                s = acts.tile([P, CH], F16, tag="s")
                nc.scalar.activation(s[:], zh[:], AF.Tanh, bias=bv[layer][:])
                return s

            first_fin_mm = [True]

            def fin_matmul(col, rhs, stop=False):
                """Accumulate W4(or 2NU*W4)^T @ rhs into fin row `m`.

                col = 128 - m for the plain-W4 column, 256 - m for 2NU*W4.
                """
                nc.tensor.matmul(
                    fin[:],
                    b4pad[:, col : col + P],
                    rhs,
                    start=first_fin_mm[0],
                    stop=stop,
                )
                first_fin_mm[0] = False

            # ---------- equation-point chunks ----------
            for c in range(NCH_EQ):
                txT = io_pool.tile([2, CH], F32, tag="txT")
                nc.sync.dma_start(
                    out=txT[:],
                    in_=tx_eq[c * CH : (c + 1) * CH, :].rearrange("n k -> k n"),
                )

                # layer 0
                s = mlp_layer_forward(txT[:], 0)
                sq = tmp.tile([P, CH], F16, tag="sq")
                nc.scalar.activation(sq[:], s[:], AF.Square)
                d = tmp.tile([P, CH], F16, tag="d")
                nc.vector.tensor_scalar(d[:], sq[:], -1.0, 1.0, ALU.mult, ALU.add)
                der = acts.tile([P, 3 * CH], F16, tag="der")
                nc.vector.tensor_scalar(der[:, 0:CH], d[:], w0t[:], None, ALU.mult)
                nc.vector.tensor_scalar(der[:, CH : 2 * CH], d[:], w0x[:], None, ALU.mult)
                sd = tmp.tile([P, CH], F16, tag="sd")
                nc.vector.tensor_tensor(sd[:], s[:], d[:], ALU.mult)
                nc.vector.tensor_scalar(
                    der[:, 2 * CH : 3 * CH], sd[:], w0xsq[:], None, ALU.mult
                )

                # layers 1..3
                for layer in (1, 2, 3):
                    zh = psum_z.tile([P, CH], F32, tag="zh")
                    nc.tensor.matmul(zh[:], wf16[layer][:], s[:], start=True, stop=True)
                    zd = psum_d.tile([P, 3 * CH], F32, tag="zd")
                    for j in range(3):
                        nc.tensor.matmul(
                            zd[:, j * CH : (j + 1) * CH],
                            wf16[layer][:],
                            der[:, j * CH : (j + 1) * CH],
                            start=True,
                            stop=True,
                        )
                    s2 = acts.tile([P, CH], F16, tag="s")
                    nc.scalar.activation(s2[:], zh[:], AF.Tanh, bias=bv[layer][:])
                    zdf = tmp.tile([P, 3 * CH], F16, tag="zdf")
                    nc.scalar.activation(zdf[:], zd[:], AF.Copy)
                    sq2 = tmp.tile([P, CH], F16, tag="sq")
                    nc.scalar.activation(sq2[:], s2[:], AF.Square)
                    d2 = tmp.tile([P, CH], F16, tag="d")
                    nc.vector.tensor_scalar(d2[:], sq2[:], -1.0, 1.0, ALU.mult, ALU.add)
                    der2 = acts.tile([P, 3 * CH], F16, tag="der")
                    # ht' = d * zt ; hx' = d * zx
                    nc.vector.tensor_tensor(der2[:, 0:CH], d2[:], zdf[:, 0:CH], ALU.mult)
                    nc.vector.tensor_tensor(
                        der2[:, CH : 2 * CH], d2[:], zdf[:, CH : 2 * CH], ALU.mult
                    )
                    # hxx~' = d*zxx~ + s*(d*zx^2)
                    q = tmp.tile([P, CH], F16, tag="q")
                    nc.vector.tensor_tensor(
                        q[:], der2[:, CH : 2 * CH], zdf[:, CH : 2 * CH], ALU.mult
                    )
                    r = tmp.tile([P, CH], F16, tag="r")
                    nc.vector.tensor_tensor(r[:], s2[:], q[:], ALU.mult)
                    a = tmp.tile([P, CH], F16, tag="a")
                    nc.vector.tensor_tensor(
                        a[:], d2[:], zdf[:, 2 * CH : 3 * CH], ALU.mult
                    )
                    nc.vector.tensor_tensor(der2[:, 2 * CH : 3 * CH], a[:], r[:], ALU.add)
                    s, der = s2, der2

                # output layer (masked stationaries -> fin rows)
                fin_matmul(P - c, s[:])                                 # u -> row c
                fin_matmul(P - (32 + c), der[:, 0:CH])                  # u_t -> row 32+c
                fin_matmul(2 * P - (32 + c), der[:, 2 * CH : 3 * CH])   # -NU*u_xx -> row 32+c
                fin_matmul(P - (64 + c), der[:, CH : 2 * CH])           # u_x -> row 64+c

            # ---------- init / boundary chunks (forward only) ----------
            for c2, tx_src in enumerate((tx_init, tx_bound)):
                txT = io_pool.tile([2, CH], F32, tag="txT")
                nc.sync.dma_start(
                    out=txT[:], in_=tx_src[0:CH, :].rearrange("n k -> k n")
                )
                h = mlp_layer_forward(txT[:], 0)
                for layer in (1, 2, 3):
                    h = mlp_layer_forward(h[:], layer)
                fin_matmul(P - (96 + c2), h[:], stop=(c2 == 1))

            # ---------- residual + outputs ----------
            u_sb = tmp.tile([32, CH], F32, tag="usb")
            nc.scalar.activation(u_sb[:], fin[0:32, :], AF.Identity, bias=b4bc[0:32])
            t1 = tmp.tile([32, CH], F32, tag="t1")
            nc.vector.tensor_tensor(t1[:], u_sb[:], fin[64:96, :], ALU.mult)
            res = tmp.tile([32, CH], F32, tag="res")
            nc.vector.tensor_tensor(res[:], t1[:], fin[32:64, :], ALU.add)
            nc.sync.dma_start(
                out=out_eq.rearrange("(c q) o -> c (q o)", c=32), in_=res[:]
            )

            uib_sb = tmp.tile([2, CH], F32, tag="uib")
            nc.scalar.activation(uib_sb[:], fin[96:98, :], AF.Identity, bias=b4bc[0:2])
            nc.sync.dma_start(
                out=out_init.rearrange("(o q) one -> o (q one)", o=1),
                in_=uib_sb[0:1, :],
            )
            nc.sync.dma_start(
                out=out_bound.rearrange("(o q) one -> o (q one)", o=1),
                in_=uib_sb[1:2, :],
            )

    nc.compile()
    return nc


_NC_CACHE = None


def _get_nc():
    global _NC_CACHE
    if _NC_CACHE is None:
        _NC_CACHE = _build_program()
    return _NC_CACHE


def kernel(
    tx_equation, tx_init, tx_bound, W0, b0, W1, b1, W2, b2, W3, b3, W4, b4, **_kw
):
    nc = _get_nc()

    def f32(x):
        return np.ascontiguousarray(np.asarray(x), dtype=np.float32)

    tx_equation = f32(tx_equation)
    tx_init = f32(tx_init)
    tx_bound = f32(tx_bound)
    weights = {
        "W0": f32(W0).reshape(2, P),
        "W1": f32(W1),
        "W2": f32(W2),
        "W3": f32(W3),
        "W4": f32(W4).reshape(P, 1),
        "b0": f32(b0).reshape(P),
        "b1": f32(b1).reshape(P),
        "b2": f32(b2).reshape(P),
        "b3": f32(b3).reshape(P),
        "b4": f32(b4).reshape(1),
    }

    in_maps = []
    for core in range(N_CORES):
        m = dict(weights)
        m["tx_eq"] = tx_equation[core * NEQC : (core + 1) * NEQC]
        m["tx_init"] = tx_init[core * NIBC : (core + 1) * NIBC]
        m["tx_bound"] = tx_bound[core * NIBC : (core + 1) * NIBC]
        in_maps.append(m)

    results = run_bass_kernel_spmd(nc, in_maps, core_ids=list(range(N_CORES))).results

    burgers = np.concatenate([r["out_eq"] for r in results], axis=0)
    u_init = np.concatenate([r["out_init"] for r in results], axis=0)
    u_bound = np.concatenate([r["out_bound"] for r in results], axis=0)
    return burgers, u_init, u_bound
